# revision 6
# baseline (speedup 1.0000x reference)
"""Bilinear-sampling + global average pooling on 8 Trainium2 NeuronCores.

Math: out[b,c] = mean_{h,w} bilinear(data[b,c], grid + ts*offset[b])
The gather indices/weights depend only on (b,h,w), never on c, so the whole
op is a linear functional over spatial positions applied per channel:

    out[b,c] = sum_s A[b,s] * data[b,c,s]      (s = flattened H*W)

where A is the scatter-accumulation of the four bilinear corner weights of
every sample point (divided by H*W).  A is computed on host from `offset`
(131K elements, 0.1% of `data`); the device kernel does the memory-bound
weighted reduction over the 128MB `data` tensor.

Device kernel (per core, 4 batches), PE/PSUM-free (fp32 matmul into PSUM
wedges this container's hardware):
  - POOL partition_broadcast replicates A[b] quarters across 128 partitions.
  - DVE tensor_tensor multiplies each [128,1024] data quarter by the
    broadcast A quarter; ACT activation(Copy, accum_out) reduces the product
    along the free dim into one column per quarter.
  - DVE tensor_reduce combines the 4 quarter columns; tiny DMA writes the
    [128] result per (batch, channel-half).
  - Sharding: data-parallel over batch, 4 batches per core.
"""

import os
import sys

import numpy as np

for _p in ("/opt/trn_rl_repo", "/root/.axon_site/_ro/trn_rl_repo"):
    if os.path.isdir(_p) and _p not in sys.path:
        sys.path.append(_p)

import concourse.bacc as bacc
import concourse.mybir as mybir
import concourse.tile as tile
from concourse.bass_utils import run_bass_kernel_spmd

N_CORES = 8
B, C, H, W = 32, 256, 64, 64
S = H * W            # 4096 spatial positions
NB = B // N_CORES    # 4 batches per core
NCH = C // 128       # 2 channel halves of 128 partitions
NQ = 4               # spatial quarters (PSUM double-buffer granularity)
QS = S // NQ         # 1024

_CACHE = {}
LAST_RESULTS = None  # BassKernelResults of the most recent run (for test.py)


def _build_nc():
    nc = bacc.Bacc("TRN2", target_bir_lowering=False, debug=False,
                   num_devices=N_CORES)
    x = nc.dram_tensor("x", [NB * NCH, 128, S], mybir.dt.float32,
                       kind="ExternalInput")
    aw = nc.dram_tensor("aw", [NB, S], mybir.dt.float32, kind="ExternalInput")
    y = nc.dram_tensor("y", [NB, NCH, 128], mybir.dt.float32,
                       kind="ExternalOutput")
    xt, at, yt = x.ap(), aw.ap(), y.ap()
    mult = mybir.AluOpType.mult
    add = mybir.AluOpType.add
    f32 = mybir.dt.float32

    with tile.TileContext(nc) as tc:
        with (
            tc.tile_pool(name="arow", bufs=2) as arowp,
            tc.tile_pool(name="abc", bufs=2 * NQ) as abcp,
            tc.tile_pool(name="data", bufs=12) as datap,
            tc.tile_pool(name="prod", bufs=4) as prodp,
            tc.tile_pool(name="junk", bufs=1) as junkp,
            tc.tile_pool(name="col", bufs=2 * NCH) as colp,
        ):
            junk = junkp.tile([128, QS], f32)

            for b in range(NB):
                a_sb = arowp.tile([1, S], f32)
                nc.sync.dma_start(a_sb[:], at[b : b + 1, :])

                abqs = []
                for q in range(NQ):
                    abq = abcp.tile([128, QS], f32)
                    nc.gpsimd.partition_broadcast(
                        abq[:], a_sb[0:1, q * QS : (q + 1) * QS])
                    abqs.append(abq)

                for ch in range(NCH):
                    col4 = colp.tile([128, NQ], f32)
                    colf = colp.tile([128, 1], f32)
                    for q in range(NQ):
                        d = datap.tile([128, QS], f32)
                        nc.sync.dma_start(
                            d[:], xt[b * NCH + ch][:, q * QS : (q + 1) * QS])
                        prod = prodp.tile([128, QS], f32)
                        nc.vector.tensor_tensor(
                            out=prod[:], in0=d[:], in1=abqs[q][:], op=mult)
                        nc.scalar.activation(
                            junk[:], prod[:],
                            mybir.ActivationFunctionType.Copy,
                            accum_out=col4[:, q : q + 1])
                    nc.vector.tensor_reduce(
                        colf[:], col4[:], axis=mybir.AxisListType.X, op=add)
                    nc.sync.dma_start(yt[b, ch], colf[:, 0])

    nc.compile()
    return nc


def _weight_field(offset, trans_std):
    """A[b,s]: accumulated bilinear weights per source pixel, incl. 1/(H*W).

    Mirrors the reference coordinate math in float32.
    """
    offset = np.asarray(offset, np.float32)
    ts = np.float32(min(max(float(trans_std), 0.001), 0.01))
    ii = np.arange(H, dtype=np.float32)[None, :, None]
    jj = np.arange(W, dtype=np.float32)[None, None, :]
    y = np.clip(ii + ts * offset[:, 0] * np.float32(H),
                np.float32(0.0), np.float32(H - 1))
    x = np.clip(jj + ts * offset[:, 1] * np.float32(W),
                np.float32(0.0), np.float32(W - 1))
    y0 = np.clip(np.floor(y).astype(np.int32), 0, H - 2)
    x0 = np.clip(np.floor(x).astype(np.int32), 0, W - 2)
    wy = (y - y0.astype(np.float32)).astype(np.float64)
    wx = (x - x0.astype(np.float32)).astype(np.float64)

    base = np.arange(offset.shape[0], dtype=np.int64)[:, None, None] * S
    i00 = (y0.astype(np.int64) * W + x0 + base).ravel()
    i01 = i00 + 1
    i10 = i00 + W
    i11 = i10 + 1
    n = offset.shape[0] * S
    acc = (
        np.bincount(i00, ((1 - wy) * (1 - wx)).ravel(), minlength=n)
        + np.bincount(i01, ((1 - wy) * wx).ravel(), minlength=n)
        + np.bincount(i10, (wy * (1 - wx)).ravel(), minlength=n)
        + np.bincount(i11, (wy * wx).ravel(), minlength=n)
    )
    return (acc / S).astype(np.float32).reshape(offset.shape[0], S)


def kernel(data, offset, trans_std):
    global LAST_RESULTS
    data = np.asarray(data, np.float32)
    offset = np.asarray(offset, np.float32)
    ts = float(np.asarray(trans_std).reshape(()))

    aw = _weight_field(offset, ts)  # [B, S] f32

    if "nc" not in _CACHE:
        _CACHE["nc"] = _build_nc()
    nc = _CACHE["nc"]

    # x shard layout: [NB*NCH, 128, S] — batch-major, channel half; the
    # device DMAs quarter-column slices of each [128, S] slab.
    xs = data.reshape(B, NCH, 128, S)
    in_maps = []
    for i in range(N_CORES):
        xi = np.ascontiguousarray(
            xs[i * NB : (i + 1) * NB].reshape(NB * NCH, 128, S))
        ai = np.ascontiguousarray(aw[i * NB : (i + 1) * NB])
        in_maps.append({"x": xi, "aw": ai})

    res = run_bass_kernel_spmd(nc, in_maps, core_ids=list(range(N_CORES)))
    LAST_RESULTS = res
    out = np.concatenate(
        [res.results[i]["y"].reshape(NB, C) for i in range(N_CORES)], axis=0)
    return np.ascontiguousarray(out.astype(np.float32))
